# revision 39
# baseline (speedup 1.0000x reference)
"""Trainium2 Bass kernel for AllGNN message passing.

Computes, for full inputs:
    h   = x @ W_in + b_in
    deg = adj.sum(axis=1, keepdims=True)
    agg = (adj @ h) / (deg + 1)
    out = agg @ W_cls + b_cls

Key algebra: row scaling commutes with the right matmul, so
    out = (adj @ G)[:, 1:] / (deg+1) + b_cls
with G = [ones | x @ W2 + b2], W2 = W_in @ W_cls, b2 = b_in @ W_cls.
The ones column's product recovers deg.

Sharding: row-shard adj over 8 cores. The adj row-block is shipped
pre-transposed (adjT = adj_blk.T, [N, rows]) and pre-cast to fp8e4 on the
host -- adj is 0/1 so fp8 is exact and HBM traffic drops 4x vs fp32.
x is shipped pre-transposed in fp8e3 (replicated, rel err ~1.6e-2); each core computes the
full G locally, interleaved with the main loop.

Main loop (adj-stationary): for each (j-tile, i-tile) pair the fp8 adjT
tile [128j, 128i] is the STATIONARY operand -- LDWEIGHTS ingests fp8 at
4 elem/cycle via FWL and hides behind the previous matmul -- and the
41-col bf16 G tile is the moving operand (41-cycle fill). Measured rate
for this shape class is ~26-30 ns per LDW+MM pair, vs 1 col/cycle
(~59 us total) when the strip is the moving operand. Output accumulates
directly as out[i, c] in PSUM: 3 banks x 4 i-tile regions [128, 48].
Bank clearing: only the first region per bank uses start=True (whole-
bank has_written clear); the other regions' first matmuls rely on
overwrite-where-unwritten.
"""

import numpy as np

import concourse.bass as bass
from concourse import bacc
import concourse.mybir as mybir
import concourse.tile as tile
from concourse.bass_utils import run_bass_kernel_spmd

import ml_dtypes

N_CORES = 8
N_NODES = 12000
IN_CH = 256
HID = 64
N_CLS = 40

ROWS = N_NODES // N_CORES        # 1500 output rows per core
ROWS_PAD = 1536                  # padded i-dim: 12 full 128-tiles (FWL needs 128)
N_IT = ROWS_PAD // 128           # 12 i-tiles
JW = 128                         # j (contraction) tile width
N_JT = -(-N_NODES // JW)         # 94 real j-tiles
N_KT = IN_CH // 128              # 2 k-tiles for x @ W2
GC = N_CLS + 1                   # G columns: [ones | g]
GJT = 8                          # j-tiles per G-production chunk
N_GCH = 12                       # G chunks
JPAD = N_GCH * GJT * JW          # 12288 padded j-dim
GRP = 8                          # j-tiles per adjT strip-group DMA
N_GRP = JPAD // (GRP * JW)       # 12 strip-group DMAs (1.57 MB each):
                                 # fewer issues through the ~10 shared DMA
                                 # sem lanes -> the tail strip lands earlier
RPB = 4                          # psum regions (i-tiles) per bank
RW = 48                          # region stride in fp32 (41 used, 16B-aligned)


def build_gnn(
    n_cores=N_CORES,
    strip_bufs=12,
    n_warmup=4,
):
    f32 = mybir.dt.float32
    bf16 = mybir.dt.bfloat16
    f8 = mybir.dt.float8e4
    f8e3 = mybir.dt.float8e3
    mult = mybir.AluOpType.mult
    add = mybir.AluOpType.add

    nc = bacc.Bacc(num_devices=n_cores)

    # adjT pre-grouped on host: [group, partition, tile-in-group, i] so each
    # partition's GRP j-tiles are contiguous (9216 B lines per partition)
    adjT_h = nc.dram_tensor(
        "adjT", [N_GRP, 128, GRP, ROWS_PAD], f8, kind="ExternalInput"
    )
    xt_h = nc.dram_tensor("x_Ti", [128, N_KT, JPAD], f8e3, kind="ExternalInput")
    # all small weights host-packed into one tensor -> one DMA:
    # [128, eye(128) | W_in as (p, t*64+h) | W_cls (64p) | b_in (64p) | b_cls bcast]
    WP_W = 128 + 128 + N_CLS + 1 + N_CLS  # 337
    wpack_h = nc.dram_tensor("wpack", [128, WP_W], f32, kind="ExternalInput")
    out_h = nc.dram_tensor("out_blk", [ROWS, N_CLS], f32, kind="ExternalOutput")

    with tile.TileContext(nc) as tc:
        with (
            tc.tile_pool(name="singles", bufs=1) as singles,
            tc.tile_pool(name="gpool", bufs=N_GCH) as g_pool,
            tc.tile_pool(name="spool", bufs=strip_bufs) as strip_pool,
            tc.tile_pool(name="opool", bufs=6) as out_pool,
            tc.tile_pool(name="psum", bufs=1, space="PSUM") as psum_pool,
        ):
            # PE warmup: junk matmuls (no DMA deps) so the HAM clock-gate
            # reaches K=8/8 before real work arrives
            wu_a = singles.tile([128, 128], bf16, tag="wu_a")
            nc.vector.memset(wu_a, 0.0)
            wu_b = singles.tile([128, 512], bf16, tag="wu_b")
            nc.vector.memset(wu_b, 0.0)
            for _ in range(n_warmup):
                ps_wu = psum_pool.tile([128, 512], f32, tag="g", bufs=3)
                nc.tensor.matmul(ps_wu, lhsT=wu_a, rhs=wu_b, start=True, stop=True)

            # one packed weight DMA, first on the sync ring (FIFO -> lands
            # before the strip-group DMAs hog the SDMA engines)
            wpack = singles.tile([128, WP_W], f32, tag="wpack")
            nc.sync.dma_start(out=wpack, in_=wpack_h[:])
            id_f = wpack[:, 0:128]
            wcls_sb = wpack[:HID, 256 : 256 + N_CLS]
            bin_sb = wpack[:HID, 296:297]
            bcls_sb = wpack[:, 297 : 297 + N_CLS]

            def win_sb(t):  # W_in k-tile [128, 64]
                return wpack[:, 128 + HID * t : 128 + HID * (t + 1)]

            # persistent PSUM banks: 3 banks x 4 regions of [128, 48] fp32
            psU = [
                psum_pool.tile([128, RPB, RW], f32, tag=f"U{i}", name=f"U{i}", bufs=1)
                for i in range(N_IT // RPB)
            ]

            # ---- Phase A: W2 = W_in @ W_cls, b2 = b_in @ W_cls (tiny) ----
            ones_sb = singles.tile([1, 128], f32, tag="ones")
            nc.vector.memset(ones_sb, 1.0)

            # W_in.T tiles via PE transpose (fp32)
            winT_sb = singles.tile([HID, N_KT, 128], f32, tag="winT")
            for t in range(N_KT):
                ps_w = psum_pool.tile([128, 512], f32, tag="g", bufs=3)
                ps = ps_w[:HID, :128]
                nc.tensor.matmul(ps, lhsT=win_sb(t), rhs=id_f, start=True, stop=True)
                nc.vector.tensor_copy(winT_sb[:, t, :], ps)
            # W2 = W_in @ W_cls -> bf16
            w2b_sb = singles.tile([128, N_KT, N_CLS], bf16, tag="w2b")
            for t in range(N_KT):
                ps_w = psum_pool.tile([128, 512], f32, tag="g", bufs=3)
                ps = ps_w[:, :N_CLS]
                nc.tensor.matmul(
                    ps, lhsT=winT_sb[:, t, :], rhs=wcls_sb, start=True, stop=True
                )
                nc.vector.tensor_copy(w2b_sb[:, t, :], ps)
            # b2 = b_in @ W_cls broadcast to [128, N_CLS]
            ps_b2w = psum_pool.tile([128, 512], f32, tag="g", bufs=3)
            ps_b2 = ps_b2w[:1, :N_CLS]
            nc.tensor.matmul(ps_b2, lhsT=bin_sb, rhs=wcls_sb, start=True, stop=True)
            b2row = singles.tile([1, N_CLS], f32, tag="b2row")
            nc.vector.tensor_copy(b2row, ps_b2)
            ps_b2bw = psum_pool.tile([128, 512], f32, tag="g", bufs=3)
            ps_b2b = ps_b2bw[:, :N_CLS]
            nc.tensor.matmul(ps_b2b, lhsT=ones_sb, rhs=b2row, start=True, stop=True)
            b2b_sb = singles.tile([128, N_CLS], f32, tag="b2b")
            nc.vector.tensor_copy(b2b_sb, ps_b2b)

            # ---- x: one persistent SBUF tile, filled by 4 piece-DMAs
            # interleaved with the adjT strips on the SAME sync ring.
            # Single-queue FIFO means a DMA's sem-lane predecessor is always
            # an earlier same-queue DMA (already drained) -> no cross-queue
            # lane-reuse stalls; interleaving keeps G production ~one strip
            # ahead of consumption.
            x_sb = singles.tile([128, N_KT, JPAD], f8e3, tag="x_sb")
            X_PC = 3 * GJT * JW  # x piece = 3 G chunks worth of j

            def x_piece(p):
                # one DMA per k-tile: per-partition contiguous lines spread
                # across all 16 SDMA engines (a [128, 2, n] slice fans over
                # only 2 engines and builds a huge per-engine backlog)
                for t in range(N_KT):
                    nc.sync.dma_start(
                        out=x_sb[:, t, p * X_PC : (p + 1) * X_PC],
                        in_=xt_h[:, t, p * X_PC : (p + 1) * X_PC],
                    )

            # ---- G production: chunk q = GJT j-tiles of [ones | x@W2 + b2],
            # covering strip groups 2q and 2q+1 (GJT == 2*GRP).
            G_tiles = {}

            def g_job(q):
                if q >= N_GCH or q in G_tiles:
                    return
                gt = g_pool.tile([128, GJT, GC], bf16, tag="G", name="G")
                nc.vector.memset(gt[:, :, 0:1], 1.0)
                for s in range(GJT):
                    ps_gw = psum_pool.tile([128, 512], f32, tag="g", bufs=3)
                    ps_g = ps_gw[:, :N_CLS]
                    for t in range(N_KT):
                        nc.tensor.matmul(
                            ps_g,
                            lhsT=x_sb[:, t, (q * GJT + s) * JW : (q * GJT + s + 1) * JW],
                            rhs=w2b_sb[:, t, :],
                            start=(t == 0),
                            stop=(t == N_KT - 1),
                        )
                    nc.vector.tensor_add(gt[:, s, 1:GC], ps_g, b2b_sb)
                G_tiles[q] = gt

            # ---- Phase B: stream adjT strip-groups; adj-stationary matmuls.
            # For each (jt, it): lhsT = fp8 adjT tile [128j, 128i] (FWL
            # ingestion, LDW hidden), rhs = G tile [128j, 41] (41-cycle
            # fill). Accumulates out[i, c] over all jt into region it%4 of
            # bank it//4. start=True only on (jt==0, it%RPB==0): clears the
            # whole bank; other regions' first matmuls overwrite-where-
            # unwritten (their has_written bits were cleared by the region-0
            # start and nothing wrote them since).
            # dedicated junk PSUM bank for HAM-warming filler matmuls
            ps_junk = psum_pool.tile([128, 512], f32, tag="junk", bufs=1)

            def finalize(it):
                # psU region (it): [128i, 41] = [deg | class sums]
                i0 = it * 128
                if i0 >= ROWS:
                    return
                p = min(128, ROWS - i0)
                ps = psU[it // RPB][:, it % RPB, :]
                d1 = out_pool.tile([128, 1], f32, tag="d1", name="d1")
                nc.vector.tensor_scalar_add(d1[:p], ps[:p, 0:1], 1.0)
                rcp = out_pool.tile([128, 1], f32, tag="rcp", name="rcp")
                nc.vector.reciprocal(rcp[:p], d1[:p])
                o_sb = out_pool.tile([128, N_CLS], f32, tag="o", name="o")
                nc.vector.scalar_tensor_tensor(
                    out=o_sb[:p],
                    in0=ps[:p, 1:GC],
                    scalar=rcp[:p],
                    in1=bcls_sb[:p],
                    op0=mult,
                    op1=add,
                )
                eng = nc.sync if it % 2 == 0 else nc.scalar
                eng.dma_start(out=out_h[i0 : i0 + p, :], in_=o_sb[:p])

            # All x pieces are enqueued BEFORE the strips: the sync queue is
            # FIFO-fed and never starves, so total DMA end-time is order-
            # independent -- but front-loading x removes the mid-stream
            # bubbles where strips paused ~2us behind an x piece. Strips then
            # arrive at a steady ~2.0us cadence that the per-group PE work
            # (~2.1us incl G share and filler) slightly exceeds: the PE stays
            # dense and warm and tracks the DMA stream to the end.
            for p in range(4):
                x_piece(p)
            # bridge the longer initial strip wait with junk so the HAM
            # clock gate stays released until real work arrives
            for _ in range(6):
                nc.tensor.matmul(
                    ps_junk[:, :256],
                    lhsT=wu_a,
                    rhs=wu_b[:, :256],
                    start=True,
                    stop=True,
                    skip_group_check=True,
                )
            g_sched = {0: 0, 2: 3, 5: 6, 8: 9}
            for g in range(N_GRP):
                if g in g_sched:
                    for q in range(g_sched[g], g_sched[g] + 3):
                        g_job(q)
                grp = strip_pool.tile([128, GRP, ROWS_PAD], f8, tag="strip")
                gw = min(GRP, N_JT - g * GRP)  # skip all-zero padded j-tiles
                nc.sync.dma_start(out=grp[:, :gw, :], in_=adjT_h[g][:, :gw, :])
                for s in range(gw):
                    jt = g * GRP + s
                    gt = G_tiles[jt // GJT]
                    gs = jt % GJT
                    for it in range(N_IT):
                        nc.tensor.matmul(
                            psU[it // RPB][:, it % RPB, :GC],
                            lhsT=grp[:, s, it * 128 : (it + 1) * 128],
                            rhs=gt[:, gs, :],
                            start=(jt == 0 and it % RPB == 0),
                            stop=(jt == N_JT - 1),
                            skip_group_check=True,
                        )
                # dependency-free filler: bridges the DMA-pacing bubble to
                # the next strip so the HAM clock gate never sees an idle
                # window and the PE stays at 2.4 GHz
                if g < N_GRP - 1:
                    nc.tensor.matmul(
                        ps_junk[:, :256],
                        lhsT=wu_a,
                        rhs=wu_b[:, :256],
                        start=True,
                        stop=True,
                        skip_group_check=True,
                    )
            for it in range(N_IT):
                finalize(it)

    nc.compile()
    return nc


_CACHE = {}


def _get_nc():
    if "nc" not in _CACHE:
        _CACHE["nc"] = build_gnn()
    return _CACHE["nc"]


def make_in_maps(x, adj, W_in, b_in, W_cls, b_cls):
    f8 = ml_dtypes.float8_e4m3
    adj8 = np.asarray(adj, dtype=np.float32).astype(f8)
    xp = np.zeros((IN_CH, JPAD), dtype=np.float32)
    xp[:, :N_NODES] = np.asarray(x, dtype=np.float32).T
    x_Ti_full = np.ascontiguousarray(
        xp.reshape(N_KT, 128, JPAD).transpose(1, 0, 2)
    ).astype(ml_dtypes.float8_e3m4)
    wpack = np.zeros((128, 128 + 128 + N_CLS + 1 + N_CLS), dtype=np.float32)
    wpack[:, 0:128] = np.eye(128, dtype=np.float32)
    wpack[:, 128:256] = (
        np.asarray(W_in, dtype=np.float32)
        .reshape(N_KT, 128, HID)
        .transpose(1, 0, 2)
        .reshape(128, N_KT * HID)
    )
    wpack[:HID, 256 : 256 + N_CLS] = np.asarray(W_cls, dtype=np.float32)
    wpack[:HID, 296] = np.asarray(b_in, dtype=np.float32)
    wpack[:, 297 : 297 + N_CLS] = np.asarray(b_cls, dtype=np.float32)[None, :]
    in_maps = []
    for c in range(N_CORES):
        sl = slice(c * ROWS, (c + 1) * ROWS)
        blk = np.zeros((JPAD, ROWS_PAD), dtype=f8)
        blk[:N_NODES, :ROWS] = adj8[sl, :].T
        blk = np.ascontiguousarray(
            blk.reshape(N_GRP, GRP, 128, ROWS_PAD).transpose(0, 2, 1, 3)
        )
        in_maps.append({"adjT": blk, "x_Ti": x_Ti_full, "wpack": wpack})
    return in_maps


def kernel(x, adj, W_in, b_in, W_cls, b_cls):
    x = np.asarray(x, dtype=np.float32)
    adj = np.asarray(adj, dtype=np.float32)
    W_in = np.asarray(W_in, dtype=np.float32)
    b_in = np.asarray(b_in, dtype=np.float32)
    W_cls = np.asarray(W_cls, dtype=np.float32)
    b_cls = np.asarray(b_cls, dtype=np.float32)

    nc = _get_nc()
    in_maps = make_in_maps(x, adj, W_in, b_in, W_cls, b_cls)
    res = run_bass_kernel_spmd(nc, in_maps, core_ids=list(range(N_CORES)))
    outs = [res.results[c]["out_blk"] for c in range(N_CORES)]
    return np.concatenate(outs, axis=0).astype(np.float32)


# revision 40
# speedup vs baseline: 1.0892x; 1.0892x over previous
"""Trainium2 Bass kernel for AllGNN message passing.

Computes, for full inputs:
    h   = x @ W_in + b_in
    deg = adj.sum(axis=1, keepdims=True)
    agg = (adj @ h) / (deg + 1)
    out = agg @ W_cls + b_cls

Key algebra: row scaling commutes with the right matmul, so
    out = (adj @ G)[:, 1:] / (deg+1) + b_cls
with G = [ones | x @ W2 + b2], W2 = W_in @ W_cls, b2 = b_in @ W_cls.
The ones column's product recovers deg.

Sharding: row-shard adj over 8 cores. The adj row-block is shipped
pre-transposed (adjT = adj_blk.T, [N, rows]) and pre-cast to fp8e4 on the
host -- adj is 0/1 so fp8 is exact and HBM traffic drops 4x vs fp32.
x is shipped pre-transposed in fp8e3 (replicated, rel err ~1.6e-2); each core computes the
full G locally, interleaved with the main loop.

Main loop (adj-stationary): for each (j-tile, i-tile) pair the fp8 adjT
tile [128j, 128i] is the STATIONARY operand -- LDWEIGHTS ingests fp8 at
4 elem/cycle via FWL and hides behind the previous matmul -- and the
41-col bf16 G tile is the moving operand (41-cycle fill). Measured rate
for this shape class is ~26-30 ns per LDW+MM pair, vs 1 col/cycle
(~59 us total) when the strip is the moving operand. Output accumulates
directly as out[i, c] in PSUM: 3 banks x 4 i-tile regions [128, 48].
Bank clearing: only the first region per bank uses start=True (whole-
bank has_written clear); the other regions' first matmuls rely on
overwrite-where-unwritten.
"""

import numpy as np

import concourse.bass as bass
from concourse import bacc
import concourse.mybir as mybir
import concourse.tile as tile
from concourse.bass_utils import run_bass_kernel_spmd

import ml_dtypes

N_CORES = 8
N_NODES = 12000
IN_CH = 256
HID = 64
N_CLS = 40

ROWS = N_NODES // N_CORES        # 1500 output rows per core
ROWS_PAD = 1536                  # padded i-dim: 12 full 128-tiles (FWL needs 128)
N_IT = ROWS_PAD // 128           # 12 i-tiles
JW = 128                         # j (contraction) tile width
N_JT = -(-N_NODES // JW)         # 94 real j-tiles
N_KT = IN_CH // 128              # 2 k-tiles for x @ W2
GC = N_CLS + 1                   # G columns: [ones | g]
GJT = 8                          # j-tiles per G-production chunk
N_GCH = 12                       # G chunks
JPAD = N_GCH * GJT * JW          # 12288 padded j-dim
GRP = 8                          # j-tiles per adjT strip-group DMA
N_GRP = JPAD // (GRP * JW)       # 12 strip-group DMAs (1.57 MB each):
                                 # fewer issues through the ~10 shared DMA
                                 # sem lanes -> the tail strip lands earlier
RPB = 4                          # psum regions (i-tiles) per bank
RW = 48                          # region stride in fp32 (41 used, 16B-aligned)


def build_gnn(
    n_cores=N_CORES,
    strip_bufs=12,
    n_warmup=4,
):
    f32 = mybir.dt.float32
    bf16 = mybir.dt.bfloat16
    f8 = mybir.dt.float8e4
    f8e3 = mybir.dt.float8e3
    mult = mybir.AluOpType.mult
    add = mybir.AluOpType.add

    nc = bacc.Bacc(num_devices=n_cores)

    # adjT pre-grouped on host: [group, partition, tile-in-group, i] so each
    # partition's GRP j-tiles are contiguous (9216 B lines per partition)
    adjT_h = nc.dram_tensor(
        "adjT", [N_GRP, 128, GRP, ROWS_PAD], f8, kind="ExternalInput"
    )
    xt_h = nc.dram_tensor("x_Ti", [128, N_KT, JPAD], f8e3, kind="ExternalInput")
    # all small weights host-packed into one tensor -> one DMA:
    # [128, eye(128) | W_in as (p, t*64+h) | W_cls (64p) | b_in (64p) | b_cls bcast]
    WP_W = 128 + 128 + N_CLS + 1 + N_CLS  # 337
    wpack_h = nc.dram_tensor("wpack", [128, WP_W], f32, kind="ExternalInput")
    # partition-major output layout: per-partition line = 12*160B = 1920B
    # contiguous -> ONE fast out-DMA (row-major [1500,40] needed 160B-line
    # descriptors that drained at ~18GB/s for ~10us). Host untangles.
    out_h = nc.dram_tensor(
        "out_blk", [128, N_IT, N_CLS], f32, kind="ExternalOutput"
    )

    with tile.TileContext(nc) as tc:
        with (
            tc.tile_pool(name="singles", bufs=1) as singles,
            tc.tile_pool(name="gpool", bufs=N_GCH) as g_pool,
            tc.tile_pool(name="spool", bufs=strip_bufs) as strip_pool,
            tc.tile_pool(name="opool", bufs=6) as out_pool,
            tc.tile_pool(name="psum", bufs=1, space="PSUM") as psum_pool,
        ):
            # PE warmup: junk matmuls (no DMA deps) so the HAM clock-gate
            # reaches K=8/8 before real work arrives
            wu_a = singles.tile([128, 128], bf16, tag="wu_a")
            nc.vector.memset(wu_a, 0.0)
            wu_b = singles.tile([128, 512], bf16, tag="wu_b")
            nc.vector.memset(wu_b, 0.0)
            for _ in range(n_warmup):
                ps_wu = psum_pool.tile([128, 512], f32, tag="g", bufs=3)
                nc.tensor.matmul(ps_wu, lhsT=wu_a, rhs=wu_b, start=True, stop=True)

            # one packed weight DMA, first on the sync ring (FIFO -> lands
            # before the strip-group DMAs hog the SDMA engines)
            wpack = singles.tile([128, WP_W], f32, tag="wpack")
            nc.sync.dma_start(out=wpack, in_=wpack_h[:])
            id_f = wpack[:, 0:128]
            wcls_sb = wpack[:HID, 256 : 256 + N_CLS]
            bin_sb = wpack[:HID, 296:297]
            bcls_sb = wpack[:, 297 : 297 + N_CLS]

            def win_sb(t):  # W_in k-tile [128, 64]
                return wpack[:, 128 + HID * t : 128 + HID * (t + 1)]

            # persistent PSUM banks: 3 banks x 4 regions of [128, 48] fp32
            psU = [
                psum_pool.tile([128, RPB, RW], f32, tag=f"U{i}", name=f"U{i}", bufs=1)
                for i in range(N_IT // RPB)
            ]

            # ---- Phase A: W2 = W_in @ W_cls, b2 = b_in @ W_cls (tiny) ----
            ones_sb = singles.tile([1, 128], f32, tag="ones")
            nc.vector.memset(ones_sb, 1.0)

            # W_in.T tiles via PE transpose (fp32)
            winT_sb = singles.tile([HID, N_KT, 128], f32, tag="winT")
            for t in range(N_KT):
                ps_w = psum_pool.tile([128, 512], f32, tag="g", bufs=3)
                ps = ps_w[:HID, :128]
                nc.tensor.matmul(ps, lhsT=win_sb(t), rhs=id_f, start=True, stop=True)
                nc.vector.tensor_copy(winT_sb[:, t, :], ps)
            # W2 = W_in @ W_cls -> bf16
            w2b_sb = singles.tile([128, N_KT, N_CLS], bf16, tag="w2b")
            for t in range(N_KT):
                ps_w = psum_pool.tile([128, 512], f32, tag="g", bufs=3)
                ps = ps_w[:, :N_CLS]
                nc.tensor.matmul(
                    ps, lhsT=winT_sb[:, t, :], rhs=wcls_sb, start=True, stop=True
                )
                nc.vector.tensor_copy(w2b_sb[:, t, :], ps)
            # b2 = b_in @ W_cls broadcast to [128, N_CLS]
            ps_b2w = psum_pool.tile([128, 512], f32, tag="g", bufs=3)
            ps_b2 = ps_b2w[:1, :N_CLS]
            nc.tensor.matmul(ps_b2, lhsT=bin_sb, rhs=wcls_sb, start=True, stop=True)
            b2row = singles.tile([1, N_CLS], f32, tag="b2row")
            nc.vector.tensor_copy(b2row, ps_b2)
            ps_b2bw = psum_pool.tile([128, 512], f32, tag="g", bufs=3)
            ps_b2b = ps_b2bw[:, :N_CLS]
            nc.tensor.matmul(ps_b2b, lhsT=ones_sb, rhs=b2row, start=True, stop=True)
            b2b_sb = singles.tile([128, N_CLS], f32, tag="b2b")
            nc.vector.tensor_copy(b2b_sb, ps_b2b)

            # ---- x: one persistent SBUF tile, filled by 4 piece-DMAs
            # interleaved with the adjT strips on the SAME sync ring.
            # Single-queue FIFO means a DMA's sem-lane predecessor is always
            # an earlier same-queue DMA (already drained) -> no cross-queue
            # lane-reuse stalls; interleaving keeps G production ~one strip
            # ahead of consumption.
            x_sb = singles.tile([128, N_KT, JPAD], f8e3, tag="x_sb")
            X_PC = 3 * GJT * JW  # x piece = 3 G chunks worth of j

            def x_piece(p):
                # one DMA per k-tile: per-partition contiguous lines spread
                # across all 16 SDMA engines (a [128, 2, n] slice fans over
                # only 2 engines and builds a huge per-engine backlog)
                for t in range(N_KT):
                    nc.sync.dma_start(
                        out=x_sb[:, t, p * X_PC : (p + 1) * X_PC],
                        in_=xt_h[:, t, p * X_PC : (p + 1) * X_PC],
                    )

            # ---- G production: chunk q = GJT j-tiles of [ones | x@W2 + b2],
            # covering strip groups 2q and 2q+1 (GJT == 2*GRP).
            G_tiles = {}

            def g_job(q):
                if q >= N_GCH or q in G_tiles:
                    return
                gt = g_pool.tile([128, GJT, GC], bf16, tag="G", name="G")
                nc.vector.memset(gt[:, :, 0:1], 1.0)
                for s in range(GJT):
                    ps_gw = psum_pool.tile([128, 512], f32, tag="g", bufs=3)
                    ps_g = ps_gw[:, :N_CLS]
                    for t in range(N_KT):
                        nc.tensor.matmul(
                            ps_g,
                            lhsT=x_sb[:, t, (q * GJT + s) * JW : (q * GJT + s + 1) * JW],
                            rhs=w2b_sb[:, t, :],
                            start=(t == 0),
                            stop=(t == N_KT - 1),
                        )
                    nc.vector.tensor_add(gt[:, s, 1:GC], ps_g, b2b_sb)
                G_tiles[q] = gt

            # ---- Phase B: stream adjT strip-groups; adj-stationary matmuls.
            # For each (jt, it): lhsT = fp8 adjT tile [128j, 128i] (FWL
            # ingestion, LDW hidden), rhs = G tile [128j, 41] (41-cycle
            # fill). Accumulates out[i, c] over all jt into region it%4 of
            # bank it//4. start=True only on (jt==0, it%RPB==0): clears the
            # whole bank; other regions' first matmuls overwrite-where-
            # unwritten (their has_written bits were cleared by the region-0
            # start and nothing wrote them since).
            # dedicated junk PSUM bank for HAM-warming filler matmuls
            ps_junk = psum_pool.tile([128, 512], f32, tag="junk", bufs=1)

            o_all = singles.tile([128, N_IT, N_CLS], f32, tag="o_all")
            nc.vector.memset(o_all, 0.0)  # pad rows must not be uninit

            def finalize(it):
                # psU region (it): [128i, 41] = [deg | class sums]
                i0 = it * 128
                p = min(128, ROWS - i0)
                ps = psU[it // RPB][:, it % RPB, :]
                d1 = out_pool.tile([128, 1], f32, tag="d1", name="d1")
                nc.vector.tensor_scalar_add(d1[:p], ps[:p, 0:1], 1.0)
                rcp = out_pool.tile([128, 1], f32, tag="rcp", name="rcp")
                nc.vector.reciprocal(rcp[:p], d1[:p])
                nc.vector.scalar_tensor_tensor(
                    out=o_all[:p, it, :],
                    in0=ps[:p, 1:GC],
                    scalar=rcp[:p],
                    in1=bcls_sb[:p],
                    op0=mult,
                    op1=add,
                )

            # All x pieces are enqueued BEFORE the strips: the sync queue is
            # FIFO-fed and never starves, so total DMA end-time is order-
            # independent -- but front-loading x removes the mid-stream
            # bubbles where strips paused ~2us behind an x piece. Strips then
            # arrive at a steady ~2.0us cadence that the per-group PE work
            # (~2.1us incl G share and filler) slightly exceeds: the PE stays
            # dense and warm and tracks the DMA stream to the end.
            for p in range(4):
                x_piece(p)
            # bridge the longer initial strip wait with junk so the HAM
            # clock gate stays released until real work arrives
            for _ in range(6):
                nc.tensor.matmul(
                    ps_junk[:, :256],
                    lhsT=wu_a,
                    rhs=wu_b[:, :256],
                    start=True,
                    stop=True,
                    skip_group_check=True,
                )
            g_sched = {0: 0, 2: 3, 5: 6, 8: 9}
            for g in range(N_GRP):
                if g in g_sched:
                    for q in range(g_sched[g], g_sched[g] + 3):
                        g_job(q)
                grp = strip_pool.tile([128, GRP, ROWS_PAD], f8, tag="strip")
                gw = min(GRP, N_JT - g * GRP)  # skip all-zero padded j-tiles
                nc.sync.dma_start(out=grp[:, :gw, :], in_=adjT_h[g][:, :gw, :])
                for s in range(gw):
                    jt = g * GRP + s
                    gt = G_tiles[jt // GJT]
                    gs = jt % GJT
                    for it in range(N_IT):
                        nc.tensor.matmul(
                            psU[it // RPB][:, it % RPB, :GC],
                            lhsT=grp[:, s, it * 128 : (it + 1) * 128],
                            rhs=gt[:, gs, :],
                            start=(jt == 0 and it % RPB == 0),
                            stop=(jt == N_JT - 1),
                            skip_group_check=True,
                        )
                # dependency-free filler: bridges the DMA-pacing bubble to
                # the next strip so the HAM clock gate never sees an idle
                # window and the PE stays at 2.4 GHz
                if g < N_GRP - 1:
                    nc.tensor.matmul(
                        ps_junk[:, :256],
                        lhsT=wu_a,
                        rhs=wu_b[:, :256],
                        start=True,
                        stop=True,
                        skip_group_check=True,
                    )
            for it in range(N_IT):
                finalize(it)
            nc.sync.dma_start(out=out_h[:], in_=o_all)

    nc.compile()
    return nc


_CACHE = {}


def _get_nc():
    if "nc" not in _CACHE:
        _CACHE["nc"] = build_gnn()
    return _CACHE["nc"]


def make_in_maps(x, adj, W_in, b_in, W_cls, b_cls):
    f8 = ml_dtypes.float8_e4m3
    adj8 = np.asarray(adj, dtype=np.float32).astype(f8)
    xp = np.zeros((IN_CH, JPAD), dtype=np.float32)
    xp[:, :N_NODES] = np.asarray(x, dtype=np.float32).T
    x_Ti_full = np.ascontiguousarray(
        xp.reshape(N_KT, 128, JPAD).transpose(1, 0, 2)
    ).astype(ml_dtypes.float8_e3m4)
    wpack = np.zeros((128, 128 + 128 + N_CLS + 1 + N_CLS), dtype=np.float32)
    wpack[:, 0:128] = np.eye(128, dtype=np.float32)
    wpack[:, 128:256] = (
        np.asarray(W_in, dtype=np.float32)
        .reshape(N_KT, 128, HID)
        .transpose(1, 0, 2)
        .reshape(128, N_KT * HID)
    )
    wpack[:HID, 256 : 256 + N_CLS] = np.asarray(W_cls, dtype=np.float32)
    wpack[:HID, 296] = np.asarray(b_in, dtype=np.float32)
    wpack[:, 297 : 297 + N_CLS] = np.asarray(b_cls, dtype=np.float32)[None, :]
    in_maps = []
    for c in range(N_CORES):
        sl = slice(c * ROWS, (c + 1) * ROWS)
        blk = np.zeros((JPAD, ROWS_PAD), dtype=f8)
        blk[:N_NODES, :ROWS] = adj8[sl, :].T
        blk = np.ascontiguousarray(
            blk.reshape(N_GRP, GRP, 128, ROWS_PAD).transpose(0, 2, 1, 3)
        )
        in_maps.append({"adjT": blk, "x_Ti": x_Ti_full, "wpack": wpack})
    return in_maps


def kernel(x, adj, W_in, b_in, W_cls, b_cls):
    x = np.asarray(x, dtype=np.float32)
    adj = np.asarray(adj, dtype=np.float32)
    W_in = np.asarray(W_in, dtype=np.float32)
    b_in = np.asarray(b_in, dtype=np.float32)
    W_cls = np.asarray(W_cls, dtype=np.float32)
    b_cls = np.asarray(b_cls, dtype=np.float32)

    nc = _get_nc()
    in_maps = make_in_maps(x, adj, W_in, b_in, W_cls, b_cls)
    res = run_bass_kernel_spmd(nc, in_maps, core_ids=list(range(N_CORES)))
    outs = []
    for c in range(N_CORES):
        blk = np.asarray(res.results[c]["out_blk"])  # [128, N_IT, N_CLS]
        outs.append(blk.transpose(1, 0, 2).reshape(-1, N_CLS)[:ROWS])
    return np.concatenate(outs, axis=0).astype(np.float32)


# revision 41
# speedup vs baseline: 1.1373x; 1.0441x over previous
"""Trainium2 Bass kernel for AllGNN message passing.

Computes, for full inputs:
    h   = x @ W_in + b_in
    deg = adj.sum(axis=1, keepdims=True)
    agg = (adj @ h) / (deg + 1)
    out = agg @ W_cls + b_cls

Key algebra: row scaling commutes with the right matmul, so
    out = (adj @ G)[:, 1:] / (deg+1) + b_cls
with G = [ones | x @ W2 + b2], W2 = W_in @ W_cls, b2 = b_in @ W_cls.
The ones column's product recovers deg.

Sharding: row-shard adj over 8 cores. The adj row-block is shipped
pre-transposed (adjT = adj_blk.T, [N, rows]) and pre-cast to fp8e4 on the
host -- adj is 0/1 so fp8 is exact and HBM traffic drops 4x vs fp32.
x is shipped pre-transposed in fp8e3 (replicated, rel err ~1.6e-2); each core computes the
full G locally, interleaved with the main loop.

Main loop (adj-stationary): for each (j-tile, i-tile) pair the fp8 adjT
tile [128j, 128i] is the STATIONARY operand -- LDWEIGHTS ingests fp8 at
4 elem/cycle via FWL and hides behind the previous matmul -- and the
41-col bf16 G tile is the moving operand (41-cycle fill). Measured rate
for this shape class is ~26-30 ns per LDW+MM pair, vs 1 col/cycle
(~59 us total) when the strip is the moving operand. Output accumulates
directly as out[i, c] in PSUM: 3 banks x 4 i-tile regions [128, 48].
Bank clearing: only the first region per bank uses start=True (whole-
bank has_written clear); the other regions' first matmuls rely on
overwrite-where-unwritten.
"""

import numpy as np

import concourse.bass as bass
from concourse import bacc
import concourse.mybir as mybir
import concourse.tile as tile
from concourse.bass_utils import run_bass_kernel_spmd

import ml_dtypes

N_CORES = 8
N_NODES = 12000
IN_CH = 256
HID = 64
N_CLS = 40

ROWS = N_NODES // N_CORES        # 1500 output rows per core
ROWS_PAD = 1536                  # padded i-dim: 12 full 128-tiles (FWL needs 128)
N_IT = ROWS_PAD // 128           # 12 i-tiles
JW = 128                         # j (contraction) tile width
N_JT = -(-N_NODES // JW)         # 94 real j-tiles
N_KT = IN_CH // 128              # 2 k-tiles for x @ W2
GC = N_CLS + 1                   # G columns: [ones | g]
GJT = 8                          # j-tiles per G-production chunk
N_GCH = 12                       # G chunks
JPAD = N_GCH * GJT * JW          # 12288 padded j-dim
GRP = 8                          # j-tiles per adjT strip-group DMA
N_GRP = JPAD // (GRP * JW)       # 12 strip-group DMAs (1.57 MB each):
                                 # fewer issues through the ~10 shared DMA
                                 # sem lanes -> the tail strip lands earlier
RPB = 4                          # psum regions (i-tiles) per bank
RW = 48                          # region stride in fp32 (41 used, 16B-aligned)


def build_gnn(
    n_cores=N_CORES,
    strip_bufs=12,
    n_warmup=4,
):
    f32 = mybir.dt.float32
    bf16 = mybir.dt.bfloat16
    f8 = mybir.dt.float8e4
    f8e3 = mybir.dt.float8e3
    mult = mybir.AluOpType.mult
    add = mybir.AluOpType.add

    nc = bacc.Bacc(num_devices=n_cores)

    # adjT pre-grouped on host: [group, partition, tile-in-group, i] so each
    # partition's GRP j-tiles are contiguous (9216 B lines per partition)
    adjT_h = nc.dram_tensor(
        "adjT", [N_GRP, 128, GRP, ROWS_PAD], f8, kind="ExternalInput"
    )
    xt_h = nc.dram_tensor("x_Ti", [128, N_KT, JPAD], f8e3, kind="ExternalInput")
    # all small weights host-packed into one tensor -> one DMA:
    # [128, eye(128) | W_in as (p, t*64+h) | W_cls (64p) | b_in (64p) | b_cls bcast]
    WP_W = 128 + 128 + N_CLS + 1 + N_CLS  # 337
    wpack_h = nc.dram_tensor("wpack", [128, WP_W], f32, kind="ExternalInput")
    # partition-major output layout: per-partition line = 12*160B = 1920B
    # contiguous -> ONE fast out-DMA (row-major [1500,40] needed 160B-line
    # descriptors that drained at ~18GB/s for ~10us). Host untangles.
    out_h = nc.dram_tensor(
        "out_blk", [128, N_IT, N_CLS], f32, kind="ExternalOutput"
    )

    with tile.TileContext(nc) as tc:
        with (
            tc.tile_pool(name="singles", bufs=1) as singles,
            tc.tile_pool(name="gpool", bufs=N_GCH) as g_pool,
            tc.tile_pool(name="spool", bufs=strip_bufs) as strip_pool,
            tc.tile_pool(name="opool", bufs=6) as out_pool,
            tc.tile_pool(name="psum", bufs=1, space="PSUM") as psum_pool,
        ):
            # PE warmup: junk matmuls (no DMA deps) so the HAM clock-gate
            # reaches K=8/8 before real work arrives
            wu_a = singles.tile([128, 128], bf16, tag="wu_a")
            nc.vector.memset(wu_a, 0.0)
            wu_b = singles.tile([128, 512], bf16, tag="wu_b")
            nc.vector.memset(wu_b, 0.0)
            for _ in range(n_warmup):
                ps_wu = psum_pool.tile([128, 512], f32, tag="g", bufs=3)
                nc.tensor.matmul(ps_wu, lhsT=wu_a, rhs=wu_b, start=True, stop=True)

            # one packed weight DMA, first on the sync ring (FIFO -> lands
            # before the strip-group DMAs hog the SDMA engines)
            wpack = singles.tile([128, WP_W], f32, tag="wpack")
            nc.sync.dma_start(out=wpack, in_=wpack_h[:])
            id_f = wpack[:, 0:128]
            wcls_sb = wpack[:HID, 256 : 256 + N_CLS]
            bin_sb = wpack[:HID, 296:297]
            bcls_sb = wpack[:, 297 : 297 + N_CLS]

            def win_sb(t):  # W_in k-tile [128, 64]
                return wpack[:, 128 + HID * t : 128 + HID * (t + 1)]

            # persistent PSUM banks: 3 banks x 4 regions of [128, 48] fp32
            psU = [
                psum_pool.tile([128, RPB, RW], f32, tag=f"U{i}", name=f"U{i}", bufs=1)
                for i in range(N_IT // RPB)
            ]

            # ---- Phase A: W2 = W_in @ W_cls, b2 = b_in @ W_cls (tiny) ----
            ones_sb = singles.tile([1, 128], f32, tag="ones")
            nc.vector.memset(ones_sb, 1.0)

            # W_in.T tiles via PE transpose (fp32)
            winT_sb = singles.tile([HID, N_KT, 128], f32, tag="winT")
            for t in range(N_KT):
                ps_w = psum_pool.tile([128, 512], f32, tag="g", bufs=3)
                ps = ps_w[:HID, :128]
                nc.tensor.matmul(ps, lhsT=win_sb(t), rhs=id_f, start=True, stop=True)
                nc.vector.tensor_copy(winT_sb[:, t, :], ps)
            # W2 = W_in @ W_cls -> bf16
            w2b_sb = singles.tile([128, N_KT, N_CLS], bf16, tag="w2b")
            for t in range(N_KT):
                ps_w = psum_pool.tile([128, 512], f32, tag="g", bufs=3)
                ps = ps_w[:, :N_CLS]
                nc.tensor.matmul(
                    ps, lhsT=winT_sb[:, t, :], rhs=wcls_sb, start=True, stop=True
                )
                nc.vector.tensor_copy(w2b_sb[:, t, :], ps)
            # b2 = b_in @ W_cls broadcast to [128, N_CLS]
            ps_b2w = psum_pool.tile([128, 512], f32, tag="g", bufs=3)
            ps_b2 = ps_b2w[:1, :N_CLS]
            nc.tensor.matmul(ps_b2, lhsT=bin_sb, rhs=wcls_sb, start=True, stop=True)
            b2row = singles.tile([1, N_CLS], f32, tag="b2row")
            nc.vector.tensor_copy(b2row, ps_b2)
            ps_b2bw = psum_pool.tile([128, 512], f32, tag="g", bufs=3)
            ps_b2b = ps_b2bw[:, :N_CLS]
            nc.tensor.matmul(ps_b2b, lhsT=ones_sb, rhs=b2row, start=True, stop=True)
            b2b_sb = singles.tile([128, N_CLS], f32, tag="b2b")
            nc.vector.tensor_copy(b2b_sb, ps_b2b)

            # ---- x: one persistent SBUF tile, filled by 4 piece-DMAs
            # interleaved with the adjT strips on the SAME sync ring.
            # Single-queue FIFO means a DMA's sem-lane predecessor is always
            # an earlier same-queue DMA (already drained) -> no cross-queue
            # lane-reuse stalls; interleaving keeps G production ~one strip
            # ahead of consumption.
            x_sb = singles.tile([128, N_KT, JPAD], f8e3, tag="x_sb")
            X_PC = 3 * GJT * JW  # x piece = 3 G chunks worth of j

            def x_piece(p):
                # one DMA per k-tile: per-partition contiguous lines spread
                # across all 16 SDMA engines (a [128, 2, n] slice fans over
                # only 2 engines and builds a huge per-engine backlog)
                for t in range(N_KT):
                    nc.sync.dma_start(
                        out=x_sb[:, t, p * X_PC : (p + 1) * X_PC],
                        in_=xt_h[:, t, p * X_PC : (p + 1) * X_PC],
                    )

            # ---- G production: chunk q = GJT j-tiles of [ones | x@W2 + b2],
            # covering strip groups 2q and 2q+1 (GJT == 2*GRP).
            G_tiles = {}

            def g_job(q):
                if q >= N_GCH or q in G_tiles:
                    return
                gt = g_pool.tile([128, GJT, GC], bf16, tag="G", name="G")
                nc.vector.memset(gt[:, :, 0:1], 1.0)
                for s in range(GJT):
                    ps_gw = psum_pool.tile([128, 512], f32, tag="g", bufs=3)
                    ps_g = ps_gw[:, :N_CLS]
                    for t in range(N_KT):
                        nc.tensor.matmul(
                            ps_g,
                            lhsT=x_sb[:, t, (q * GJT + s) * JW : (q * GJT + s + 1) * JW],
                            rhs=w2b_sb[:, t, :],
                            start=(t == 0),
                            stop=(t == N_KT - 1),
                        )
                    nc.vector.tensor_add(gt[:, s, 1:GC], ps_g, b2b_sb)
                G_tiles[q] = gt

            # ---- Phase B: stream adjT strip-groups; adj-stationary matmuls.
            # For each (jt, it): lhsT = fp8 adjT tile [128j, 128i] (FWL
            # ingestion, LDW hidden), rhs = G tile [128j, 41] (41-cycle
            # fill). Accumulates out[i, c] over all jt into region it%4 of
            # bank it//4. start=True only on (jt==0, it%RPB==0): clears the
            # whole bank; other regions' first matmuls overwrite-where-
            # unwritten (their has_written bits were cleared by the region-0
            # start and nothing wrote them since).
            # dedicated junk PSUM bank for HAM-warming filler matmuls
            ps_junk = psum_pool.tile([128, 512], f32, tag="junk", bufs=1)

            o_all = singles.tile([128, N_IT, N_CLS], f32, tag="o_all")
            nc.vector.memset(o_all, 0.0)  # pad rows must not be uninit

            def finalize_bank(b):
                # batch deg+1 and reciprocal across the bank's 4 regions
                # (one DVE op each instead of 4), then per-region STT; the
                # bank's out-DMA pipelines behind the next bank's DVE work
                d1 = out_pool.tile([128, RPB, 1], f32, tag="d1", name="d1")
                nc.vector.tensor_scalar_add(d1, psU[b][:, :, 0:1], 1.0)
                rcp = out_pool.tile([128, RPB, 1], f32, tag="rcp", name="rcp")
                nc.vector.reciprocal(rcp, d1)
                for r in range(RPB):
                    it = b * RPB + r
                    p = min(128, ROWS - it * 128)
                    nc.vector.scalar_tensor_tensor(
                        out=o_all[:p, it, :],
                        in0=psU[b][:p, r, 1:GC],
                        scalar=rcp[:p, r, :],
                        in1=bcls_sb[:p],
                        op0=mult,
                        op1=add,
                    )
                nc.sync.dma_start(
                    out=out_h[:, b * RPB : (b + 1) * RPB, :],
                    in_=o_all[:, b * RPB : (b + 1) * RPB, :],
                )

            # All x pieces are enqueued BEFORE the strips: the sync queue is
            # FIFO-fed and never starves, so total DMA end-time is order-
            # independent -- but front-loading x removes the mid-stream
            # bubbles where strips paused ~2us behind an x piece. Strips then
            # arrive at a steady ~2.0us cadence that the per-group PE work
            # (~2.1us incl G share and filler) slightly exceeds: the PE stays
            # dense and warm and tracks the DMA stream to the end.
            for p in range(4):
                x_piece(p)
            # bridge the longer initial strip wait with junk so the HAM
            # clock gate stays released until real work arrives
            for _ in range(6):
                nc.tensor.matmul(
                    ps_junk[:, :256],
                    lhsT=wu_a,
                    rhs=wu_b[:, :256],
                    start=True,
                    stop=True,
                    skip_group_check=True,
                )
            g_sched = {0: 0, 2: 3, 5: 6, 8: 9}
            for g in range(N_GRP):
                if g in g_sched:
                    for q in range(g_sched[g], g_sched[g] + 3):
                        g_job(q)
                grp = strip_pool.tile([128, GRP, ROWS_PAD], f8, tag="strip")
                gw = min(GRP, N_JT - g * GRP)  # skip all-zero padded j-tiles
                nc.sync.dma_start(out=grp[:, :gw, :], in_=adjT_h[g][:, :gw, :])
                for s in range(gw):
                    jt = g * GRP + s
                    gt = G_tiles[jt // GJT]
                    gs = jt % GJT
                    for it in range(N_IT):
                        nc.tensor.matmul(
                            psU[it // RPB][:, it % RPB, :GC],
                            lhsT=grp[:, s, it * 128 : (it + 1) * 128],
                            rhs=gt[:, gs, :],
                            start=(jt == 0 and it % RPB == 0),
                            stop=(jt == N_JT - 1),
                            skip_group_check=True,
                        )
                # dependency-free filler: bridges the DMA-pacing bubble to
                # the next strip so the HAM clock gate never sees an idle
                # window and the PE stays at 2.4 GHz
                if g < N_GRP - 1:
                    nc.tensor.matmul(
                        ps_junk[:, :256],
                        lhsT=wu_a,
                        rhs=wu_b[:, :256],
                        start=True,
                        stop=True,
                        skip_group_check=True,
                    )
            for b in range(N_IT // RPB):
                finalize_bank(b)

    nc.compile()
    return nc


_CACHE = {}


def _get_nc():
    if "nc" not in _CACHE:
        _CACHE["nc"] = build_gnn()
    return _CACHE["nc"]


def make_in_maps(x, adj, W_in, b_in, W_cls, b_cls):
    f8 = ml_dtypes.float8_e4m3
    adj8 = np.asarray(adj, dtype=np.float32).astype(f8)
    xp = np.zeros((IN_CH, JPAD), dtype=np.float32)
    xp[:, :N_NODES] = np.asarray(x, dtype=np.float32).T
    x_Ti_full = np.ascontiguousarray(
        xp.reshape(N_KT, 128, JPAD).transpose(1, 0, 2)
    ).astype(ml_dtypes.float8_e3m4)
    wpack = np.zeros((128, 128 + 128 + N_CLS + 1 + N_CLS), dtype=np.float32)
    wpack[:, 0:128] = np.eye(128, dtype=np.float32)
    wpack[:, 128:256] = (
        np.asarray(W_in, dtype=np.float32)
        .reshape(N_KT, 128, HID)
        .transpose(1, 0, 2)
        .reshape(128, N_KT * HID)
    )
    wpack[:HID, 256 : 256 + N_CLS] = np.asarray(W_cls, dtype=np.float32)
    wpack[:HID, 296] = np.asarray(b_in, dtype=np.float32)
    wpack[:, 297 : 297 + N_CLS] = np.asarray(b_cls, dtype=np.float32)[None, :]
    in_maps = []
    for c in range(N_CORES):
        sl = slice(c * ROWS, (c + 1) * ROWS)
        blk = np.zeros((JPAD, ROWS_PAD), dtype=f8)
        blk[:N_NODES, :ROWS] = adj8[sl, :].T
        blk = np.ascontiguousarray(
            blk.reshape(N_GRP, GRP, 128, ROWS_PAD).transpose(0, 2, 1, 3)
        )
        in_maps.append({"adjT": blk, "x_Ti": x_Ti_full, "wpack": wpack})
    return in_maps


def kernel(x, adj, W_in, b_in, W_cls, b_cls):
    x = np.asarray(x, dtype=np.float32)
    adj = np.asarray(adj, dtype=np.float32)
    W_in = np.asarray(W_in, dtype=np.float32)
    b_in = np.asarray(b_in, dtype=np.float32)
    W_cls = np.asarray(W_cls, dtype=np.float32)
    b_cls = np.asarray(b_cls, dtype=np.float32)

    nc = _get_nc()
    in_maps = make_in_maps(x, adj, W_in, b_in, W_cls, b_cls)
    res = run_bass_kernel_spmd(nc, in_maps, core_ids=list(range(N_CORES)))
    outs = []
    for c in range(N_CORES):
        blk = np.asarray(res.results[c]["out_blk"])  # [128, N_IT, N_CLS]
        outs.append(blk.transpose(1, 0, 2).reshape(-1, N_CLS)[:ROWS])
    return np.concatenate(outs, axis=0).astype(np.float32)


# revision 42
# speedup vs baseline: 1.1604x; 1.0203x over previous
"""Trainium2 Bass kernel for AllGNN message passing.

Computes, for full inputs:
    h   = x @ W_in + b_in
    deg = adj.sum(axis=1, keepdims=True)
    agg = (adj @ h) / (deg + 1)
    out = agg @ W_cls + b_cls

Key algebra: row scaling commutes with the right matmul, so
    out = (adj @ G)[:, 1:] / (deg+1) + b_cls
with G = [ones | x @ W2 + b2], W2 = W_in @ W_cls, b2 = b_in @ W_cls.
The ones column's product recovers deg.

Sharding: row-shard adj over 8 cores. The adj row-block is shipped
pre-transposed (adjT = adj_blk.T, [N, rows]) and pre-cast to fp8e4 on the
host -- adj is 0/1 so fp8 is exact and HBM traffic drops 4x vs fp32.
x is shipped pre-transposed in fp8e3 (replicated, rel err ~1.6e-2); each core computes the
full G locally, interleaved with the main loop.

Main loop (adj-stationary): for each (j-tile, i-tile) pair the fp8 adjT
tile [128j, 128i] is the STATIONARY operand -- LDWEIGHTS ingests fp8 at
4 elem/cycle via FWL and hides behind the previous matmul -- and the
41-col bf16 G tile is the moving operand (41-cycle fill). Measured rate
for this shape class is ~26-30 ns per LDW+MM pair, vs 1 col/cycle
(~59 us total) when the strip is the moving operand. Output accumulates
directly as out[i, c] in PSUM: 3 banks x 4 i-tile regions [128, 48].
Bank clearing: only the first region per bank uses start=True (whole-
bank has_written clear); the other regions' first matmuls rely on
overwrite-where-unwritten.
"""

import numpy as np

import concourse.bass as bass
from concourse import bacc
import concourse.mybir as mybir
import concourse.tile as tile
from concourse.bass_utils import run_bass_kernel_spmd

import ml_dtypes

N_CORES = 8
N_NODES = 12000
IN_CH = 256
HID = 64
N_CLS = 40

ROWS = N_NODES // N_CORES        # 1500 output rows per core
ROWS_PAD = 1536                  # padded i-dim: 12 full 128-tiles (FWL needs 128)
N_IT = ROWS_PAD // 128           # 12 i-tiles
JW = 128                         # j (contraction) tile width
N_JT = -(-N_NODES // JW)         # 94 real j-tiles
N_KT = IN_CH // 128              # 2 k-tiles for x @ W2
GC = N_CLS + 1                   # G columns: [ones | g]
GJT = 8                          # j-tiles per G-production chunk
N_GCH = 12                       # G chunks
JPAD = N_GCH * GJT * JW          # 12288 padded j-dim
GRP = 8                          # j-tiles per adjT strip-group DMA
N_GRP = JPAD // (GRP * JW)       # 12 strip-group DMAs (1.57 MB each):
                                 # fewer issues through the ~10 shared DMA
                                 # sem lanes -> the tail strip lands earlier
RPB = 4                          # psum regions (i-tiles) per bank
RW = 48                          # region stride in fp32 (41 used, 16B-aligned)


def build_gnn(
    n_cores=N_CORES,
    strip_bufs=12,
    n_warmup=4,
):
    f32 = mybir.dt.float32
    bf16 = mybir.dt.bfloat16
    f8 = mybir.dt.float8e4
    f8e3 = mybir.dt.float8e3
    mult = mybir.AluOpType.mult
    add = mybir.AluOpType.add

    nc = bacc.Bacc(num_devices=n_cores)

    # adjT pre-grouped on host: [group, partition, tile-in-group, i] so each
    # partition's GRP j-tiles are contiguous (9216 B lines per partition)
    adjT_h = nc.dram_tensor(
        "adjT", [N_GRP, 128, GRP, ROWS_PAD], f8, kind="ExternalInput"
    )
    xt_h = nc.dram_tensor("x_Ti", [128, N_KT, JPAD], f8e3, kind="ExternalInput")
    # all small weights host-packed into one tensor -> one DMA:
    # [128, eye(128) | W_in as (p, t*64+h) | W_cls (64p) | b_in (64p) | b_cls bcast]
    WP_W = 128 + 128 + N_CLS + 1 + N_CLS  # 337
    wpack_h = nc.dram_tensor("wpack", [128, WP_W], f32, kind="ExternalInput")
    # partition-major output layout: per-partition line = 12*160B = 1920B
    # contiguous -> ONE fast out-DMA (row-major [1500,40] needed 160B-line
    # descriptors that drained at ~18GB/s for ~10us). Host untangles.
    out_h = nc.dram_tensor(
        "out_blk", [128, N_IT, N_CLS], f32, kind="ExternalOutput"
    )

    with tile.TileContext(nc) as tc:
        with (
            tc.tile_pool(name="singles", bufs=1) as singles,
            tc.tile_pool(name="gpool", bufs=N_GCH) as g_pool,
            tc.tile_pool(name="spool", bufs=strip_bufs) as strip_pool,
            tc.tile_pool(name="opool", bufs=6) as out_pool,
            tc.tile_pool(name="psum", bufs=1, space="PSUM") as psum_pool,
        ):
            # PE warmup: junk matmuls (no DMA deps) so the HAM clock-gate
            # reaches K=8/8 before real work arrives
            wu_a = singles.tile([128, 128], bf16, tag="wu_a")
            nc.vector.memset(wu_a, 0.0)
            wu_b = singles.tile([128, 512], bf16, tag="wu_b")
            nc.vector.memset(wu_b, 0.0)
            for _ in range(n_warmup):
                ps_wu = psum_pool.tile([128, 512], f32, tag="g", bufs=3)
                nc.tensor.matmul(ps_wu, lhsT=wu_a, rhs=wu_b, start=True, stop=True)

            # one packed weight DMA, first on the sync ring (FIFO -> lands
            # before the strip-group DMAs hog the SDMA engines)
            wpack = singles.tile([128, WP_W], f32, tag="wpack")
            nc.sync.dma_start(out=wpack, in_=wpack_h[:])
            id_f = wpack[:, 0:128]
            wcls_sb = wpack[:HID, 256 : 256 + N_CLS]
            bin_sb = wpack[:HID, 296:297]
            bcls_sb = wpack[:, 297 : 297 + N_CLS]

            def win_sb(t):  # W_in k-tile [128, 64]
                return wpack[:, 128 + HID * t : 128 + HID * (t + 1)]

            # persistent PSUM banks: 3 banks x 4 regions of [128, 48] fp32
            psU = [
                psum_pool.tile([128, RPB, RW], f32, tag=f"U{i}", name=f"U{i}", bufs=1)
                for i in range(N_IT // RPB)
            ]

            # ---- Phase A: W2 = W_in @ W_cls, b2 = b_in @ W_cls (tiny) ----
            ones_sb = singles.tile([1, 128], f32, tag="ones")
            nc.vector.memset(ones_sb, 1.0)

            # W_in.T tiles via PE transpose (fp32)
            winT_sb = singles.tile([HID, N_KT, 128], f32, tag="winT")
            for t in range(N_KT):
                ps_w = psum_pool.tile([128, 512], f32, tag="g", bufs=3)
                ps = ps_w[:HID, :128]
                nc.tensor.matmul(ps, lhsT=win_sb(t), rhs=id_f, start=True, stop=True)
                nc.vector.tensor_copy(winT_sb[:, t, :], ps)
            # W2 = W_in @ W_cls -> bf16
            w2b_sb = singles.tile([128, N_KT, N_CLS], bf16, tag="w2b")
            for t in range(N_KT):
                ps_w = psum_pool.tile([128, 512], f32, tag="g", bufs=3)
                ps = ps_w[:, :N_CLS]
                nc.tensor.matmul(
                    ps, lhsT=winT_sb[:, t, :], rhs=wcls_sb, start=True, stop=True
                )
                nc.vector.tensor_copy(w2b_sb[:, t, :], ps)
            # b2 = b_in @ W_cls broadcast to [128, N_CLS]
            ps_b2w = psum_pool.tile([128, 512], f32, tag="g", bufs=3)
            ps_b2 = ps_b2w[:1, :N_CLS]
            nc.tensor.matmul(ps_b2, lhsT=bin_sb, rhs=wcls_sb, start=True, stop=True)
            b2row = singles.tile([1, N_CLS], f32, tag="b2row")
            nc.vector.tensor_copy(b2row, ps_b2)
            ps_b2bw = psum_pool.tile([128, 512], f32, tag="g", bufs=3)
            ps_b2b = ps_b2bw[:, :N_CLS]
            nc.tensor.matmul(ps_b2b, lhsT=ones_sb, rhs=b2row, start=True, stop=True)
            b2b_sb = singles.tile([128, N_CLS], f32, tag="b2b")
            nc.vector.tensor_copy(b2b_sb, ps_b2b)

            # ---- x: one persistent SBUF tile, filled by 4 piece-DMAs
            # interleaved with the adjT strips on the SAME sync ring.
            # Single-queue FIFO means a DMA's sem-lane predecessor is always
            # an earlier same-queue DMA (already drained) -> no cross-queue
            # lane-reuse stalls; interleaving keeps G production ~one strip
            # ahead of consumption.
            x_sb = singles.tile([128, N_KT, JPAD], f8e3, tag="x_sb")
            X_PC = 3 * GJT * JW  # x piece = 3 G chunks worth of j

            def x_piece(p):
                # one DMA per k-tile: per-partition contiguous lines spread
                # across all 16 SDMA engines (a [128, 2, n] slice fans over
                # only 2 engines and builds a huge per-engine backlog)
                for t in range(N_KT):
                    nc.sync.dma_start(
                        out=x_sb[:, t, p * X_PC : (p + 1) * X_PC],
                        in_=xt_h[:, t, p * X_PC : (p + 1) * X_PC],
                    )

            # ---- G production: chunk q = GJT j-tiles of [ones | x@W2 + b2],
            # covering strip groups 2q and 2q+1 (GJT == 2*GRP).
            G_tiles = {}

            def g_job(q):
                if q >= N_GCH or q in G_tiles:
                    return
                gt = g_pool.tile([128, GJT, GC], bf16, tag="G", name="G")
                nc.vector.memset(gt[:, :, 0:1], 1.0)
                for s in range(GJT):
                    ps_gw = psum_pool.tile([128, 512], f32, tag="g", bufs=3)
                    ps_g = ps_gw[:, :N_CLS]
                    for t in range(N_KT):
                        nc.tensor.matmul(
                            ps_g,
                            lhsT=x_sb[:, t, (q * GJT + s) * JW : (q * GJT + s + 1) * JW],
                            rhs=w2b_sb[:, t, :],
                            start=(t == 0),
                            stop=(t == N_KT - 1),
                        )
                    nc.vector.tensor_add(gt[:, s, 1:GC], ps_g, b2b_sb)
                G_tiles[q] = gt

            # ---- Phase B: stream adjT strip-groups; adj-stationary matmuls.
            # For each (jt, it): lhsT = fp8 adjT tile [128j, 128i] (FWL
            # ingestion, LDW hidden), rhs = G tile [128j, 41] (41-cycle
            # fill). Accumulates out[i, c] over all jt into region it%4 of
            # bank it//4. start=True only on (jt==0, it%RPB==0): clears the
            # whole bank; other regions' first matmuls overwrite-where-
            # unwritten (their has_written bits were cleared by the region-0
            # start and nothing wrote them since).
            # dedicated junk PSUM bank for HAM-warming filler matmuls
            ps_junk = psum_pool.tile([128, 512], f32, tag="junk", bufs=1)

            o_all = singles.tile([128, N_IT, N_CLS], f32, tag="o_all")
            nc.vector.memset(o_all, 0.0)  # pad rows must not be uninit

            def finalize_bank(b):
                # batch deg+1 and reciprocal across the bank's 4 regions
                # (one DVE op each instead of 4), then per-region STT; the
                # bank's out-DMA pipelines behind the next bank's DVE work
                d1 = out_pool.tile([128, RPB, 1], f32, tag="d1", name="d1")
                nc.vector.tensor_scalar_add(d1, psU[b][:, :, 0:1], 1.0)
                rcp = out_pool.tile([128, RPB, 1], f32, tag="rcp", name="rcp")
                nc.vector.reciprocal(rcp, d1)
                for r in range(RPB):
                    it = b * RPB + r
                    p = min(128, ROWS - it * 128)
                    nc.vector.scalar_tensor_tensor(
                        out=o_all[:p, it, :],
                        in0=psU[b][:p, r, 1:GC],
                        scalar=rcp[:p, r, :],
                        in1=bcls_sb[:p],
                        op0=mult,
                        op1=add,
                    )
                nc.sync.dma_start(
                    out=out_h[:, b * RPB : (b + 1) * RPB, :],
                    in_=o_all[:, b * RPB : (b + 1) * RPB, :],
                )

            # All x pieces are enqueued BEFORE the strips: the sync queue is
            # FIFO-fed and never starves, so total DMA end-time is order-
            # independent -- but front-loading x removes the mid-stream
            # bubbles where strips paused ~2us behind an x piece. Strips then
            # arrive at a steady ~2.0us cadence that the per-group PE work
            # (~2.1us incl G share and filler) slightly exceeds: the PE stays
            # dense and warm and tracks the DMA stream to the end.
            for p in range(4):
                x_piece(p)
            # bridge the longer initial strip wait with junk so the HAM
            # clock gate stays released until real work arrives
            for _ in range(6):
                nc.tensor.matmul(
                    ps_junk[:, :256],
                    lhsT=wu_a,
                    rhs=wu_b[:, :256],
                    start=True,
                    stop=True,
                    skip_group_check=True,
                )
            g_sched = {0: 0, 2: 3, 5: 6, 8: 9}
            for g in range(N_GRP):
                if g in g_sched:
                    for q in range(g_sched[g], g_sched[g] + 3):
                        g_job(q)
                grp = strip_pool.tile([128, GRP, ROWS_PAD], f8, tag="strip")
                gw = min(GRP, N_JT - g * GRP)  # skip all-zero padded j-tiles
                if g == N_GRP - 1:
                    # split the final group's DMA so its first j-tiles (and
                    # their matmuls) land ~2us before the full tail arrives
                    h = gw // 2
                    nc.sync.dma_start(out=grp[:, :h, :], in_=adjT_h[g][:, :h, :])
                    nc.sync.dma_start(
                        out=grp[:, h:gw, :], in_=adjT_h[g][:, h:gw, :]
                    )
                else:
                    nc.sync.dma_start(out=grp[:, :gw, :], in_=adjT_h[g][:, :gw, :])
                for s in range(gw):
                    jt = g * GRP + s
                    gt = G_tiles[jt // GJT]
                    gs = jt % GJT
                    for it in range(N_IT):
                        nc.tensor.matmul(
                            psU[it // RPB][:, it % RPB, :GC],
                            lhsT=grp[:, s, it * 128 : (it + 1) * 128],
                            rhs=gt[:, gs, :],
                            start=(jt == 0 and it % RPB == 0),
                            stop=(jt == N_JT - 1),
                            skip_group_check=True,
                        )
                # dependency-free filler: bridges the DMA-pacing bubble to
                # the next strip so the HAM clock gate never sees an idle
                # window and the PE stays at 2.4 GHz
                if g < N_GRP - 1:
                    nc.tensor.matmul(
                        ps_junk[:, :256],
                        lhsT=wu_a,
                        rhs=wu_b[:, :256],
                        start=True,
                        stop=True,
                        skip_group_check=True,
                    )
            for b in range(N_IT // RPB):
                finalize_bank(b)

    nc.compile()
    return nc


_CACHE = {}


def _get_nc():
    if "nc" not in _CACHE:
        _CACHE["nc"] = build_gnn()
    return _CACHE["nc"]


def make_in_maps(x, adj, W_in, b_in, W_cls, b_cls):
    f8 = ml_dtypes.float8_e4m3
    adj8 = np.asarray(adj, dtype=np.float32).astype(f8)
    xp = np.zeros((IN_CH, JPAD), dtype=np.float32)
    xp[:, :N_NODES] = np.asarray(x, dtype=np.float32).T
    x_Ti_full = np.ascontiguousarray(
        xp.reshape(N_KT, 128, JPAD).transpose(1, 0, 2)
    ).astype(ml_dtypes.float8_e3m4)
    wpack = np.zeros((128, 128 + 128 + N_CLS + 1 + N_CLS), dtype=np.float32)
    wpack[:, 0:128] = np.eye(128, dtype=np.float32)
    wpack[:, 128:256] = (
        np.asarray(W_in, dtype=np.float32)
        .reshape(N_KT, 128, HID)
        .transpose(1, 0, 2)
        .reshape(128, N_KT * HID)
    )
    wpack[:HID, 256 : 256 + N_CLS] = np.asarray(W_cls, dtype=np.float32)
    wpack[:HID, 296] = np.asarray(b_in, dtype=np.float32)
    wpack[:, 297 : 297 + N_CLS] = np.asarray(b_cls, dtype=np.float32)[None, :]
    in_maps = []
    for c in range(N_CORES):
        sl = slice(c * ROWS, (c + 1) * ROWS)
        blk = np.zeros((JPAD, ROWS_PAD), dtype=f8)
        blk[:N_NODES, :ROWS] = adj8[sl, :].T
        blk = np.ascontiguousarray(
            blk.reshape(N_GRP, GRP, 128, ROWS_PAD).transpose(0, 2, 1, 3)
        )
        in_maps.append({"adjT": blk, "x_Ti": x_Ti_full, "wpack": wpack})
    return in_maps


def kernel(x, adj, W_in, b_in, W_cls, b_cls):
    x = np.asarray(x, dtype=np.float32)
    adj = np.asarray(adj, dtype=np.float32)
    W_in = np.asarray(W_in, dtype=np.float32)
    b_in = np.asarray(b_in, dtype=np.float32)
    W_cls = np.asarray(W_cls, dtype=np.float32)
    b_cls = np.asarray(b_cls, dtype=np.float32)

    nc = _get_nc()
    in_maps = make_in_maps(x, adj, W_in, b_in, W_cls, b_cls)
    res = run_bass_kernel_spmd(nc, in_maps, core_ids=list(range(N_CORES)))
    outs = []
    for c in range(N_CORES):
        blk = np.asarray(res.results[c]["out_blk"])  # [128, N_IT, N_CLS]
        outs.append(blk.transpose(1, 0, 2).reshape(-1, N_CLS)[:ROWS])
    return np.concatenate(outs, axis=0).astype(np.float32)
